# revision 1
# baseline (speedup 1.0000x reference)
"""Trainium2 Bass kernel for Conv2D-FastKAN + BatchNorm2d (training-mode stats).

Math (reference): unfold x [8,16,64,64] into patches p [N=32768, D=144] (3x3,
pad 1), RBF basis exp(-((p-g_k)/h)^2) over G=8 grids -> [N, 1152], out = basis
@ Wsum.T with Wsum = W.sum(axis=1) [32, 1152], then BatchNorm2d with batch
statistics and affine gamma/beta.

Distribution: data-parallel over batch across 8 cores (1 image each). W is
sharded over output channels (4 each) for the Wsum reduction, AllGathered;
BN statistics AllReduced.

Key device-side structure per core:
 - basis "image" computed once on the padded 66x66 grid as an SBUF tile
   [128 = (c,g), 4356]; the 9 unfold taps are strided windows of this tile
   fed directly to the TensorEngine as moving-operand access patterns.
 - contraction order k' = ((kh*3+kw)*16 + c)*8 + g so each 128-row k-chunk is
   one kernel tap; Wsum is produced directly in this order by strided matmul
   access patterns over the natural W layout.
 - float32r matmuls (full-rate fp32 storage).
"""

import numpy as np

import concourse.bacc as bacc
import concourse.bass as bass
import concourse.mybir as mybir
import concourse.tile as tile
import concourse.bass_utils as bass_utils

F32 = mybir.dt.float32
F32R = mybir.dt.float32r
AF = mybir.ActivationFunctionType
ALU = mybir.AluOpType

N_CORES = 8
B, C, H, WD = 8, 16, 64, 64
O, G = 32, 8
D = C * 9            # 144
DG = D * G           # 1152
HP = H + 2           # 66 padded
SP = HP * HP         # 4356
NLOC = H * WD         # 4096 patches per core
NTOT = B * NLOC      # 32768
O_SH = O // N_CORES  # 4 output channels of W per core
INV_H = 7.0 / 4.0    # 1/h, h = (2-(-2))/(G-1)
BN_EPS = 1e-5

_CACHE = {}


def _build(taps=False, sim=False):
    nc = bacc.Bacc("TRN2", target_bir_lowering=False, debug=False,
                   num_devices=N_CORES)

    x_b = nc.dram_tensor("x_b", [C, H, WD], F32, kind="ExternalInput").ap()
    w_b = nc.dram_tensor("w_b", [O_SH, D, DG], F32R, kind="ExternalInput").ap()
    cst = nc.dram_tensor("cst", [128, 5], F32, kind="ExternalInput").ap()
    sel = nc.dram_tensor("sel", [128, 128], F32, kind="ExternalInput").ap()
    onew = nc.dram_tensor("onew", [128, 1], F32R, kind="ExternalInput").ap()
    y_b = nc.dram_tensor("y_b", [O, H * WD], F32, kind="ExternalOutput").ap()
    if taps:
        tp_bt = nc.dram_tensor("tp_bt", [128, SP], F32, kind="ExternalOutput").ap()
        tp_wt = nc.dram_tensor("tp_wt", [128, 9 * O], F32, kind="ExternalOutput").ap()
        tp_op = nc.dram_tensor("tp_op", [O, NLOC], F32, kind="ExternalOutput").ap()

    with tile.TileContext(nc) as tc:
        with (
            tc.tile_pool(name="const", bufs=1) as cpool,
            tc.tile_pool(name="work", bufs=1) as wpool,
            tc.tile_pool(name="wld", bufs=2) as wldpool,
            tc.tile_pool(name="small", bufs=1) as spool,
            tc.tile_pool(name="psum", bufs=2, space="PSUM") as pp,
            tc.tile_pool(name="psum1", bufs=2, space="PSUM") as pp1,
            tc.tile_pool(name="dram", bufs=1, space="DRAM") as dram,
        ):
            cst_sb = cpool.tile([128, 5], F32)
            nc.sync.dma_start(cst_sb[:], cst)
            sel_sb = cpool.tile([128, 128], F32)
            nc.sync.dma_start(sel_sb[:], sel)
            onew_sb = cpool.tile([128, 1], F32R)
            nc.sync.dma_start(onew_sb[:], onew)
            warm = spool.tile([1, 1], F32)
            nc.scalar.activation(warm[:], cst_sb[0:1, 4:5], AF.Exp)

            # ---- Wsum phase: wsum_mine[o, k'] = sum_d W[o, d, col(k')] ----
            wsum_mine = dram.tile([O_SH, DG], F32)
            ws_sb = spool.tile([1, O_SH * DG], F32)
            ones_ap = onew_sb[:]
            for o in range(O_SH):
                wn0 = wldpool.tile([128, DG], F32R, tag="wn0")
                nc.sync.dma_start(wn0[:], w_b[o, 0:128, :])
                wn1 = wldpool.tile([16, DG], F32R, tag="wn1")
                nc.sync.dma_start(wn1[:], w_b[o, 128:144, :])
                for jg in range(3):
                    ps = pp1.tile([1, 384], F32, tag="wsps")
                    # moving cols iterate (jj, g, c): addr = jj*8 + g + c*72
                    rhs0 = bass.AP(wn0[:].tensor, wn0[:].offset + jg * 24,
                                   [wn0[:].ap[0], [8, 3], [1, 8], [72, 16]])
                    nc.tensor.matmul(ps[:], ones_ap, rhs0,
                                     start=True, stop=False)
                    rhs1 = bass.AP(wn1[:].tensor, wn1[:].offset + jg * 24,
                                   [wn1[:].ap[0], [8, 3], [1, 8], [72, 16]])
                    nc.tensor.matmul(ps[:], ones_ap[0:16, :],
                                     rhs1, start=False, stop=True)
                    base = (o * 3 + jg) * 384
                    nc.vector.tensor_copy(ws_sb[:, base:base + 384], ps[:])
            nc.sync.dma_start(wsum_mine[:].rearrange("o k -> (o k)")
                              .unsqueeze(0), ws_sb[:])

            # ---- AllGather Wsum: [4, 1152] x8 -> [32, 1152] in k' order ----
            wsum_all = dram.tile([O, DG], F32)
            if sim:
                nc.sync.dma_start(wsum_all[0:O_SH, :], wsum_mine[:])
            else:
                nc.gpsimd.collective_compute(
                    "AllGather", ALU.bypass,
                    replica_groups=[list(range(N_CORES))],
                    ins=[wsum_mine[:].opt()], outs=[wsum_all[:].opt()],
                )
            # -> SBUF [128 = k'-within-chunk, (j, o)] via 9 PE transposes
            wsg = cpool.tile([O, DG], F32)
            nc.sync.dma_start(wsg[:], wsum_all[:])
            wsumT = cpool.tile([128, 9 * O], F32R)
            ident = sel_sb[0:32, 0:32]
            for j in range(9):
                ps_t = pp1.tile([128, O], F32, tag="pst")
                nc.tensor.transpose(ps_t[:], wsg[:, j * 128:(j + 1) * 128],
                                    ident)
                nc.vector.tensor_copy(wsumT[:, j * O:(j + 1) * O], ps_t[:])

            # ---- basis image: Bt[(c,g), s] over padded 66x66 grid ----
            q = wpool.tile([128, SP], F32, tag="q")
            # zero halo strips (rows 0 and 65; cols 0 and 65)
            nc.vector.memset(q[:, 0:HP], 0.0)
            nc.vector.memset(q[:, (HP - 1) * HP:SP], 0.0)
            col0 = bass.AP(q[:].tensor, q[:].offset + HP,
                           [q[:].ap[0], [HP, H], [1, 1]])
            nc.vector.memset(col0, 0.0)
            col1 = bass.AP(q[:].tensor, q[:].offset + HP + HP - 1,
                           [q[:].ap[0], [HP, H], [1, 1]])
            nc.vector.memset(col1, 0.0)
            # interior: replicate x image into the 8 g-blocks
            interior = [[HP, H], [1, WD]]
            for g in range(G):
                dst_g = bass.AP(q[:].tensor,
                                q[:].offset + (g * C) * SP + HP + 1,
                                [[SP, C]] + interior)
                nc.sync.dma_start(dst_g, x_b[:, :, :])
            # two overlapping halves (rows 0..36 / 32..66) so nt=0..3
            # matmuls start as soon as the first Exp half lands
            CA, CB0 = 36 * HP, 32 * HP
            t_a = wpool.tile([128, CA], F32, tag="tsqa")
            nc.scalar.activation(t_a[:], q[:, 0:CA], AF.Square,
                                 bias=cst_sb[:, 0:1], scale=INV_H)
            bt_a = wpool.tile([128, CA], F32R, tag="bta")
            nc.scalar.activation(bt_a[:], t_a[:], AF.Exp, scale=-1.0)
            t_b = wpool.tile([128, SP - CB0], F32, tag="tsqb")
            nc.scalar.activation(t_b[:], q[:, CB0:SP], AF.Square,
                                 bias=cst_sb[:, 0:1], scale=INV_H)
            bt_b = wpool.tile([128, SP - CB0], F32R, tag="btb")
            nc.scalar.activation(bt_b[:], t_b[:], AF.Exp, scale=-1.0)

            # ---- main matmul: out_T[o, n] with partition = channel ----
            out_pack = wpool.tile([O, NLOC], F32, tag="opack")
            sts = spool.tile([O, 2], F32)
            stq = spool.tile([O, 8], F32)
            for nt in range(8):
                ps_o = pp.tile([O, 512], F32, tag="pso")
                bt_h = bt_a[:] if nt < 4 else bt_b[:]
                row0 = 8 * nt if nt < 4 else 8 * nt - 32
                for j in range(9):
                    kh, kw = j // 3, j % 3
                    rhs = bass.AP(bt_h.tensor, bt_h.offset
                                  + (row0 + kh) * HP + kw,
                                  [bt_h.ap[0], [HP, 8], [1, WD]])
                    nc.tensor.matmul(
                        ps_o[:], wsumT[:, j * O:(j + 1) * O],
                        rhs, start=(j == 0), stop=(j == 8))
                dst = out_pack[:, nt * 512:(nt + 1) * 512]
                nc.vector.tensor_copy(dst, ps_o[:])
                # squared sum on ACT (square values themselves are discarded)
                scr = wldpool.tile([O, 512], F32, tag="scr")
                nc.scalar.activation(scr[:], ps_o[:], AF.Square,
                                     accum_out=stq[:, nt:nt + 1])
                if nt == 3:
                    nc.vector.reduce_sum(sts[:, 0:1], out_pack[:, 0:2048],
                                         axis=mybir.AxisListType.X)
                if nt == 7:
                    nc.vector.reduce_sum(sts[:, 1:2], out_pack[:, 2048:4096],
                                         axis=mybir.AxisListType.X)

            if taps:
                nc.sync.dma_start(tp_bt, bt[:].bitcast(F32))
                nc.sync.dma_start(tp_wt, wsumT[:].bitcast(F32))
                nc.sync.dma_start(tp_op, out_pack[:])

            # ---- BN stats: fold per-tile partials, AllReduce over cores ----
            st = spool.tile([O, 2], F32)
            sq = wpool.tile([O, NLOC], F32, tag="sq")
            nc.vector.reduce_sum(st[:, 0:1], sts[:], axis=mybir.AxisListType.X)
            nc.vector.reduce_sum(st[:, 1:2], stq[:], axis=mybir.AxisListType.X)

            stat_in = dram.tile([O, 2], F32)
            stat_out = dram.tile([O, 2], F32)
            nc.sync.dma_start(stat_in[:], st[:])
            if sim:
                nc.sync.dma_start(stat_out[:], stat_in[:])
            else:
                nc.gpsimd.collective_compute(
                    "AllReduce", ALU.add,
                    replica_groups=[list(range(N_CORES))],
                    ins=[stat_in[:].opt()], outs=[stat_out[:].opt()],
                )
            gst = spool.tile([O, 2], F32)
            nc.sync.dma_start(gst[:], stat_out[:])

            # ---- scale/shift per channel ----
            mean = spool.tile([O, 1], F32)
            nc.scalar.mul(mean[:], gst[:, 0:1], 1.0 / NTOT)
            msq = spool.tile([O, 1], F32)
            nc.scalar.mul(msq[:], gst[:, 1:2], 1.0 / NTOT)
            var = spool.tile([O, 1], F32)
            nc.vector.tensor_mul(var[:], mean[:], mean[:])
            nc.vector.tensor_sub(var[:], msq[:], var[:])
            lnv = spool.tile([O, 1], F32)
            nc.scalar.activation(lnv[:], var[:], AF.Ln, bias=cst_sb[0:O, 4:5])
            a_t = spool.tile([O, 1], F32)
            nc.scalar.activation(a_t[:], lnv[:], AF.Exp, scale=-0.5)
            nc.vector.tensor_mul(a_t[:], a_t[:], cst_sb[0:O, 1:2])
            b_t = spool.tile([O, 1], F32)
            nc.vector.tensor_mul(b_t[:], a_t[:], mean[:])
            nc.vector.tensor_sub(b_t[:], cst_sb[0:O, 2:3], b_t[:])

            # ---- affine + output ----
            nc.vector.tensor_scalar(sq[:], out_pack[:], a_t[:, 0:1],
                                    b_t[:, 0:1], ALU.mult, ALU.add)
            nc.sync.dma_start(y_b, sq[:])

    nc.compile()
    return nc


def _host_consts():
    cst = np.zeros((128, 5), dtype=np.float32)
    r = np.arange(128)
    cst[:, 0] = -(r // 16 - 3.5)        # -g'_g for rows (g,c)
    cst[:, 3] = 1.0                     # ones column for Wsum reduction
    cst[:, 4] = BN_EPS
    sel = np.zeros((128, 128), dtype=np.float32)
    p = np.arange(128)
    sel[:, :] = (p[:, None] % 32 == p[None, :] % 32).astype(np.float32)
    return cst, sel


def kernel(x, W, gamma, beta, taps=False):
    key = ("nc", taps)
    if key not in _CACHE:
        _CACHE[key] = _build(taps)
    nc = _CACHE[key]

    cst, sel = _host_consts()
    cst = cst.copy()
    cst[:, 1] = np.asarray(gamma, np.float32)[np.arange(128) % 32]
    cst[:, 2] = np.asarray(beta, np.float32)[np.arange(128) % 32]

    x = np.ascontiguousarray(np.asarray(x, np.float32))
    W = np.ascontiguousarray(np.asarray(W, np.float32))
    in_maps = []
    for c in range(N_CORES):
        in_maps.append({
            "x_b": x[c],
            "w_b": W[c * O_SH:(c + 1) * O_SH],
            "cst": cst,
            "sel": sel,
            "onew": np.ones((128, 1), np.float32),
        })
    res = bass_utils.run_bass_kernel_spmd(nc, in_maps,
                                          core_ids=list(range(N_CORES)))
    out = np.empty((B, O, H, WD), dtype=np.float32)
    for c in range(N_CORES):
        out[c] = res.results[c]["y_b"].reshape(O, H, WD)
    if taps:
        return out, res.results
    return out

